# revision 33
# baseline (speedup 1.0000x reference)
"""Tensor-parallel GQA attention kernel for one TRN2 chip (8 NeuronCores).

Problem: hidden [1, 2048, 4096] -> q/k/v proj -> interleaved RoPE -> causal
GQA attention (32 q heads, 8 kv heads, head_dim 128) -> o_proj -> [1, 2048, 4096].

Sharding: tensor-parallel over heads. Core c owns q heads 4c..4c+3 and kv
head c. o_proj is head-sharded too: each core computes, per 512-row seq
chunk, the partial o_proj contribution of its own 4 heads (wo rows for
those heads only), and a per-chunk ReduceScatter(add) over the 8 cores
sums the partials and leaves each core with 64 rows of the final output.
All ReduceScatters except the last overlap attention compute of the
following chunks.

Device scheme (everything transposed, [feature, seq]):
  - qT/kT computed as [d, s] via matmul(lhsT=W_tile, rhs=hT_tile); RoPE in
    rotate-half form (wq/wk columns de-interleaved on host). 1/sqrt(dh)
    folded into wq on host.
  - scoresT [t, s] = matmul(lhsT=kT_tile, rhs=q_chunk); Exp on ScalarE
    evacuates PSUM->SBUF bf16; causal masking post-exp on the single
    [128,128] partial-triangle sub-block; diagonal t-tiles with di>=1 are
    column-truncated (cols < di*128 are fully masked -> skipped).
  - attn@v accumulates [d, s] with lhsT=v_tile; softmax sums via DVE
    pairwise adds then a PE ones-matmul partition-reduce, reciprocal on
    [1, 512], rank-1 PE broadcast, one DVE multiply. Each head's
    finalize (den/bcast/divide) is deferred into the NEXT head's
    attention loop as interleave thunks so its DVE latency hides under
    PE work.
  - partial o_proj: out[s_tile, hid] += o_sbT[dh, s_tile].T @ wo_h[dh, hid]
    accumulated over the 4 local heads; evacuated bf16 and DMA'd to the
    ReduceScatter input buffer.
  - creation order interleaves q-projection and next-chunk k/v projection
    matmuls into the attention tt-loops (sc(tt+1) and thunks emitted
    before av(tt)) so the PE has work while ScalarE exps lag.
  - DMA queues: loads ride the Sync queue, stores the ScalarE queue, the
    RS output copy the GpSimd queue, so nothing head-of-line-blocks.
  - ht is staged in quarter-chunk tiles and the next chunk's quarters are
    prefetched right after att(0) so the transfers land before the
    previous chunk's ReduceScatter saturates the DMA engines.
"""

import sys

if "/opt/trn_rl_repo" not in sys.path:
    sys.path.insert(0, "/opt/trn_rl_repo")

import numpy as np
import ml_dtypes

import concourse.bass as bass
import concourse.bacc as bacc
import concourse.mybir as mybir
import concourse.tile as tile
from concourse import bass_utils
from concourse.masks import make_identity

F32 = mybir.dt.float32
BF16 = mybir.dt.bfloat16
NPBF16 = ml_dtypes.bfloat16

S = 2048          # sequence length
HID = 4096        # hidden size
NH = 32           # q heads
NKV = 8           # kv heads
DH = 128          # head dim
G = NH // NKV     # q heads per kv head (= per core)
NCORES = 8
CH = 512          # attention s-chunk width
NCH = S // CH     # 4 chunks
KT = HID // 128   # 32 hidden k-tiles
QKT = KT // 4     # k-tiles per ht quarter-tile
SC2 = CH // NCORES  # seq rows per core per chunk after RS (= 64)

_CACHED = {}


def build_kernel():
    nc = bacc.Bacc("TRN2", target_bir_lowering=False, debug=False,
                   num_devices=NCORES)

    ht_d = nc.declare_dram_parameter("ht", [NCH, HID, CH], BF16, isOutput=False)
    wq_d = nc.declare_dram_parameter("wq", [G, 128, KT * 128], BF16, isOutput=False)
    wk_d = nc.declare_dram_parameter("wk", [128, KT * 128], BF16, isOutput=False)
    wv_d = nc.declare_dram_parameter("wv", [128, KT * 128], BF16, isOutput=False)
    wo_d = nc.declare_dram_parameter("wo", [G, 128, HID], BF16, isOutput=False)
    cos2_d = nc.declare_dram_parameter("cos2", [128, S], BF16, isOutput=False)
    sins_d = nc.declare_dram_parameter("sins", [128, S], BF16, isOutput=False)
    mask_d = nc.declare_dram_parameter("mask", [4, 128, CH], BF16, isOutput=False)
    out_d = nc.declare_dram_parameter("out", [NCH, SC2, HID], BF16, isOutput=True)

    with tile.TileContext(nc) as tc:
        with (
            tc.tile_pool(name="const", bufs=1) as constp,
            tc.tile_pool(name="dram", bufs=1, space="DRAM") as dramp,
            tc.tile_pool(name="wp", bufs=1) as wp,
            tc.tile_pool(name="htp", bufs=7) as htp,
            tc.tile_pool(name="kvp", bufs=1) as kvp,
            tc.tile_pool(name="psP", bufs=3, space="PSUM") as psP,
            tc.tile_pool(name="psB", bufs=3, space="PSUM") as psB,
            tc.tile_pool(name="psO", bufs=2, space="PSUM") as psO,
            tc.tile_pool(name="ropep", bufs=2) as ropep,
            tc.tile_pool(name="qcp", bufs=4) as qcp,
            tc.tile_pool(name="probp", bufs=6) as probp,
            tc.tile_pool(name="smallp", bufs=2) as smallp,
            tc.tile_pool(name="tinyp", bufs=2) as tinyp,
            tc.tile_pool(name="aoutp", bufs=6) as aoutp,
            tc.tile_pool(name="evacp", bufs=3) as evacp,
        ):
            rs_ins = [dramp.tile([CH, HID], BF16, name=f"rsi{j}")
                      for j in range(NCH - 1)]
            rs_outs = [dramp.tile([SC2, HID], BF16, name=f"rso{j}")
                       for j in range(NCH - 1)]
            # last chunk splits into sub-chunks so the final
            # ReduceScatters pipeline against the remaining compute
            SUBW = [256, 128, 128]
            rs3_ins = [dramp.tile([w, HID], BF16, name=f"rs3i{i}")
                       for i, w in enumerate(SUBW)]
            rs3_outs = [dramp.tile([w // NCORES, HID], BF16, name=f"rs3o{i}")
                        for i, w in enumerate(SUBW)]

            _htn = [0]

            def load_ht_quarter(scb, q):
                _htn[0] += 1
                t = htp.tile([128, QKT * CH], BF16,
                             name=f"ht{scb}_{q}_{_htn[0]}", tag="ht")
                kb = QKT // 2
                for b in range(2):
                    nc.sync.dma_start(
                        t[:, b * kb * CH:(b + 1) * kb * CH].rearrange(
                            "p (kt c) -> p kt c", c=CH),
                        ht_d[scb,
                             (q * QKT + b * kb) * 128:
                             (q * QKT + (b + 1) * kb) * 128].rearrange(
                            "(kt p) c -> p kt c", p=128))
                return t

            def load_ht(scb):
                """Four [128, QKT*CH] quarter tiles for seq chunk scb."""
                return [load_ht_quarter(scb, q) for q in range(4)]

            # ---- prologue: weights + first ht chunk + constants ----
            # order follows first-use: wk, the first ht quarters, then wv
            # (v-proj trails k-proj by ~7us)
            wk_t = wp.tile([128, KT * 128], BF16, name="wk", tag="wk")
            nc.sync.dma_start(wk_t[:], wk_d[:])
            wv_t = wp.tile([128, KT * 128], BF16, name="wv", tag="wv")
            ht_pre = [load_ht_quarter(0, 0), load_ht_quarter(0, 1)]
            nc.sync.dma_start(wv_t[:], wv_d[:])
            ht_pre += [load_ht_quarter(0, 2), load_ht_quarter(0, 3)]
            cos2 = constp.tile([128, S], BF16, tag="cos2")
            nc.sync.dma_start(cos2[:], cos2_d[:])
            sins = constp.tile([128, S], BF16, tag="sins")
            nc.sync.dma_start(sins[:], sins_d[:])
            wq_ts = []
            for h in range(G):
                wq_t = wp.tile([128, KT * 128], BF16, name=f"wq{h}",
                               tag=f"wq{h}")
                nc.sync.dma_start(wq_t[:], wq_d[h])
                wq_ts.append(wq_t)
            masks = constp.tile([128, 4 * CH], BF16, tag="masks")
            for i in range(4):
                nc.sync.dma_start(masks[:, i * CH:(i + 1) * CH], mask_d[i])
            ident = constp.tile([128, 128], BF16, tag="ident")
            make_identity(nc, ident[:])
            # warm the PE HAM clock-gate with junk matmuls while the
            # prologue DMAs land (idle-time otherwise)
            junk = psB.tile([128, 128], F32, tag="sc", name="junk")
            for _ in range(40):
                nc.tensor.matmul(junk[:], ident[:], ident[:],
                                 start=True, stop=True)
            ones_col = constp.tile([128, 1], F32, tag="onesc")
            nc.vector.memset(ones_col[:], 1.0)
            ones_row = constp.tile([1, 128], F32, tag="onesr")
            nc.vector.memset(ones_row[:], 1.0)
            wo_ts = []
            for h in range(G):
                wo_t = wp.tile([128, HID], BF16, name=f"wo{h}", tag=f"wo{h}")
                nc.sync.dma_start(wo_t[:], wo_d[h])
                wo_ts.append(wo_t)

            kT = kvp.tile([128, S], BF16, tag="kT")
            v_sb = kvp.tile([128, S], BF16, tag="v")  # s-tile st at [st*128,..)

            def rope_evac(ps, dst_slice, off):
                # dst[0:64] = x1*c - x2*s ; dst[64:128] = x1*s + x2*c
                # cos2 = [c; c], sins = [s; -s]; bf16 for DVE speed
                qf = ropep.tile([128, CH], BF16, tag="qf")
                nc.vector.tensor_copy(qf[:], ps[:])
                ra = ropep.tile([128, CH], BF16, tag="ra")
                nc.vector.tensor_tensor(ra[:], qf[:], cos2[:, off:off + CH],
                                        op=mybir.AluOpType.mult)
                rb = ropep.tile([128, CH], BF16, tag="rb")
                nc.vector.tensor_tensor(rb[0:64, :], qf[64:128, :],
                                        sins[64:128, off:off + CH],
                                        op=mybir.AluOpType.mult)
                nc.vector.tensor_tensor(rb[64:128, :], qf[0:64, :],
                                        sins[0:64, off:off + CH],
                                        op=mybir.AluOpType.mult)
                nc.vector.tensor_tensor(dst_slice, ra[:], rb[:],
                                        op=mybir.AluOpType.add)

            def emit_proj_mms(specs, ht_quads, kt0, kt1):
                for kt in range(kt0, kt1):
                    ht_t = ht_quads[kt // QKT]
                    col = (kt % QKT) * CH
                    for w_t, ps in specs:
                        nc.tensor.matmul(
                            ps[:], w_t[:, kt * 128:(kt + 1) * 128],
                            ht_t[:, col:col + CH],
                            start=(kt == 0), stop=(kt == KT - 1))

            def proj_thunks(specs, ht_quads, per):
                thunks = []
                for kt0 in range(0, KT, per):
                    thunks.append(lambda kt0=kt0: emit_proj_mms(
                        specs, ht_quads, kt0, min(kt0 + per, KT)))
                return thunks

            def weave(fins, thunks):
                """fin thunks at drain slots 1,3 so the producing DVE
                chains get a head start."""
                out = []
                t = list(thunks)
                for i, f in enumerate(fins):
                    if t:
                        out.append(t.pop(0))
                    out.append(f)
                return out + t

            def vproj_finish(ps_v, scb):
                vT_sb = ropep.tile([128, CH], BF16, tag="ra")
                nc.vector.tensor_copy(vT_sb[:], ps_v[:])
                for q4 in range(CH // 128):
                    st = scb * (CH // 128) + q4
                    ps_tr = psB.tile([128, 128], BF16, tag="sc")
                    nc.tensor.transpose(
                        ps_tr[:], vT_sb[:, q4 * 128:(q4 + 1) * 128], ident[:])
                    nc.vector.tensor_copy(
                        v_sb[:, st * 128:(st + 1) * 128], ps_tr[:])

            def attention(h, cb, W, qo, qc, interleave):
                """Attention for head h over s-cols [cb, cb+W) (qc offset
                qo). Returns (o_sb, fins): o_sb is allocated now but
                written by fins[1]; fins are PE thunks to interleave into
                the next head's loop."""
                nt = (cb + W) // 128
                att_ps = psO.tile([128, CH], F32, tag="att")
                acc0 = smallp.tile([128, CH], F32, tag="acc0")
                il = list(interleave)
                ili = 0
                st8 = [False]
                pend = []

                def emit_sc(tt):
                    di = tt - cb // 128
                    lo = di * 128 if di >= 1 else 0
                    sc = psB.tile([128, CH], F32, tag="sc")
                    nc.tensor.matmul(sc[:, lo:W],
                                     kT[:, tt * 128:(tt + 1) * 128],
                                     qc[:, qo + lo:qo + W],
                                     start=True, stop=True)
                    pr = probp.tile([128, CH], BF16, tag="pr")
                    nc.scalar.activation(
                        pr[:, lo:W], sc[:, lo:W],
                        mybir.ActivationFunctionType.Exp)
                    if di >= 0:
                        mo = di * CH + di * 128
                        nc.vector.tensor_tensor(
                            pr[:, di * 128:(di + 1) * 128],
                            pr[:, di * 128:(di + 1) * 128],
                            masks[:, mo:mo + 128],
                            op=mybir.AluOpType.mult)
                    return pr, lo

                def acc_add(pr, lo):
                    if lo > 0:
                        nc.vector.tensor_tensor(
                            acc0[:, lo:W], acc0[:, lo:W], pr[:, lo:W],
                            op=mybir.AluOpType.add)
                        return
                    if not st8[0] and cb == 0:
                        nc.vector.tensor_copy(acc0[:, 0:W], pr[:, 0:W])
                        st8[0] = True
                        return
                    pend.append(pr)
                    if len(pend) == 2:
                        pp = smallp.tile([128, CH], BF16, tag="pp", bufs=3)
                        nc.vector.tensor_tensor(
                            pp[:, 0:W], pend[0][:, 0:W], pend[1][:, 0:W],
                            op=mybir.AluOpType.add)
                        if not st8[0]:
                            nc.vector.tensor_copy(acc0[:, 0:W], pp[:, 0:W])
                            st8[0] = True
                        else:
                            nc.vector.tensor_tensor(
                                acc0[:, 0:W], acc0[:, 0:W], pp[:, 0:W],
                                op=mybir.AluOpType.add)
                        pend.clear()

                prs = [emit_sc(0)]
                for tt in range(nt):
                    if tt + 1 < nt:
                        prs.append(emit_sc(tt + 1))
                    if ili < len(il):
                        il[ili]()
                        ili += 1
                    pr, lo = prs[tt]
                    nc.tensor.matmul(att_ps[:, lo:W],
                                     v_sb[:, tt * 128:(tt + 1) * 128],
                                     pr[:, lo:W],
                                     start=(tt == 0), stop=(tt == nt - 1))
                    acc_add(pr, lo)
                while ili < len(il):
                    il[ili]()
                    ili += 1
                if pend:
                    if not st8[0]:
                        nc.vector.tensor_copy(acc0[:, 0:W], pend[0][:, 0:W])
                        st8[0] = True
                    else:
                        nc.vector.tensor_tensor(
                            acc0[:, 0:W], acc0[:, 0:W], pend[0][:, 0:W],
                            op=mybir.AluOpType.add)
                    pend.clear()

                o_sb = aoutp.tile([128, CH], BF16, tag="o", name=f"o{h}")
                env = {}

                def fin1():
                    bc = psB.tile([128, CH], F32, tag="sc", name="bc")
                    nc.tensor.matmul(bc[0:1, 0:W], ones_col[:, 0:1],
                                     acc0[:, 0:W], start=True, stop=True)
                    den_sb = tinyp.tile([1, CH], F32, tag="den")
                    nc.vector.tensor_copy(den_sb[0:1, 0:W], bc[0:1, 0:W])
                    rc1 = tinyp.tile([1, CH], F32, tag="rc")
                    nc.vector.reciprocal_approx_fast(
                        out=rc1[0:1, 0:W], in_=den_sb[0:1, 0:W])
                    o_raw = smallp.tile([128, CH], BF16, tag="oraw")
                    nc.vector.tensor_copy(o_raw[:, 0:W], att_ps[:, 0:W])
                    env["bc"], env["rc1"], env["o_raw"] = bc, rc1, o_raw

                def fin2():
                    bc, rc1, o_raw = env["bc"], env["rc1"], env["o_raw"]
                    nc.tensor.matmul(bc[:, 0:W], ones_row[0:1, :],
                                     rc1[0:1, 0:W], start=True, stop=True)
                    nc.vector.tensor_tensor(o_sb[:, 0:W], o_raw[:, 0:W],
                                            bc[:, 0:W],
                                            op=mybir.AluOpType.mult)

                return o_sb, [fin1, fin2]

            # ---------------- main loop over seq chunks ----------------
            ht_t = ht_pre
            pending_rs = [None]  # (scb,) of a deferred ReduceScatter

            def emit_rs(j):
                nc.gpsimd.collective_compute(
                    "ReduceScatter",
                    mybir.AluOpType.add,
                    replica_groups=[list(range(NCORES))],
                    ins=[rs_ins[j].opt()],
                    outs=[rs_outs[j].opt()],
                )
                nc.gpsimd.dma_start(out_d[j], rs_outs[j][:])

            for scb in range(NCH):
                j = scb
                nt = (CH // 128) * (j + 1)
                per = max(1, KT // nt)

                # prefetch next chunk's hidden slice at the TOP of the
                # chunk so it rides the sync queue ahead of this chunk's
                # evac stores and lands before any ReduceScatter traffic
                ht_next = load_ht(scb + 1) if scb + 1 < NCH else None
                if pending_rs[0] is not None:
                    # gate the deferred ReduceScatter on this prefetch so
                    # its SDMA traffic never delays the ht transfers; a
                    # gpsimd COMPUTE op (not a DMA trigger) so the engine
                    # genuinely blocks on the ht-load semaphores
                    if ht_next is not None:
                        kb2 = (QKT // 2) * CH
                        gate = tinyp.tile([1, CH], BF16, tag="gate",
                                          name=f"gate{scb}")
                        for q in range(4):
                            nc.gpsimd.tensor_tensor(
                                gate[0:1, q * 4:q * 4 + 4],
                                ht_next[q][0:1, kb2 - 2:kb2 + 2],
                                ht_next[q][0:1, kb2 - 2:kb2 + 2],
                                op=mybir.AluOpType.mult)
                    emit_rs(pending_rs[0])
                    pending_rs[0] = None

                if scb == 0:
                    # k first (fewest DMA deps), v second
                    ps_k = psP.tile([128, CH], F32, tag="proj")
                    emit_proj_mms([(wk_t, ps_k)], ht_t, 0, KT)
                    ps_v = psP.tile([128, CH], F32, tag="proj")
                    emit_proj_mms([(wv_t, ps_v)], ht_t, 0, KT)
                    rope_evac(ps_k, kT[:, 0:CH], 0)
                    vproj_finish(ps_v, 0)

                # q projections for heads 0,1 up front
                ps_q0 = psP.tile([128, CH], F32, tag="proj")
                ps_q1 = psP.tile([128, CH], F32, tag="proj")
                emit_proj_mms([(wq_ts[0], ps_q0), (wq_ts[1], ps_q1)],
                              ht_t, 0, KT)
                qcs = [None] * G
                qcs[0] = qcp.tile([128, CH], BF16, tag="qc", name="qc0")
                rope_evac(ps_q0, qcs[0][:], scb * CH)
                qcs[1] = qcp.tile([128, CH], BF16, tag="qc", name="qc1")
                rope_evac(ps_q1, qcs[1][:], scb * CH)

                ev_n = [0]

                def opart_pair(o_list, st, hcp, dst):
                    g1 = psP.tile([128, CH], F32, tag="proj", name="g1")
                    g2 = psP.tile([128, CH], F32, tag="proj", name="g2")
                    for h in range(G):
                        nc.tensor.matmul(
                            g1[:],
                            o_list[h][:, st * 128:(st + 1) * 128],
                            wo_ts[h][:, (2 * hcp) * CH:(2 * hcp + 1) * CH],
                            start=(h == 0), stop=(h == G - 1))
                        nc.tensor.matmul(
                            g2[:],
                            o_list[h][:, st * 128:(st + 1) * 128],
                            wo_ts[h][:, (2 * hcp + 1) * CH:(2 * hcp + 2) * CH],
                            start=(h == 0), stop=(h == G - 1))
                    stage = evacp.tile([128, 2 * CH], BF16, tag="ev")
                    if ev_n[0] % 2 == 0:
                        nc.vector.tensor_copy(stage[:, 0:CH], g1[:])
                        nc.scalar.activation(
                            stage[:, CH:2 * CH], g2[:],
                            mybir.ActivationFunctionType.Copy)
                    else:
                        nc.scalar.activation(
                            stage[:, 0:CH], g1[:],
                            mybir.ActivationFunctionType.Copy)
                        nc.vector.tensor_copy(stage[:, CH:2 * CH], g2[:])
                    ev_n[0] += 1
                    nc.sync.dma_start(
                        dst[st * 128:(st + 1) * 128,
                            hcp * 2 * CH:(hcp + 1) * 2 * CH],
                        stage[:])

                if scb < NCH - 1:
                    # head 0 attention, interleaving head 2's projection
                    ps_q2 = psP.tile([128, CH], F32, tag="proj")
                    o0, fin0 = attention(
                        0, scb * CH, CH, 0, qcs[0],
                        proj_thunks([(wq_ts[2], ps_q2)], ht_t, per))
                    qcs[2] = qcp.tile([128, CH], BF16, tag="qc", name="qc2")
                    rope_evac(ps_q2, qcs[2][:], scb * CH)

                    # head 1 attention, interleaving head 3's projection
                    # and head 0's finalize
                    ps_q3 = psP.tile([128, CH], F32, tag="proj")
                    o1, fin1 = attention(
                        1, scb * CH, CH, 0, qcs[1],
                        weave(fin0,
                              proj_thunks([(wq_ts[3], ps_q3)], ht_t, per)))
                    qcs[3] = qcp.tile([128, CH], BF16, tag="qc", name="qc3")
                    rope_evac(ps_q3, qcs[3][:], scb * CH)

                    # heads 2,3 attention, interleaving next chunk's
                    # k/v projection
                    ps_k2 = psP.tile([128, CH], F32, tag="proj", name="psk2")
                    ps_v2 = psP.tile([128, CH], F32, tag="proj", name="psv2")
                    il2 = proj_thunks([(wk_t, ps_k2)], ht_next, per)
                    il3 = proj_thunks([(wv_t, ps_v2)], ht_next, per)
                    o2, fin2 = attention(2, scb * CH, CH, 0, qcs[2],
                                         weave(fin1, il2))
                    o3, fin3 = attention(3, scb * CH, CH, 0, qcs[3],
                                         weave(fin2, il3))
                    # finalize head 3 around the next chunk's k rope
                    # (DVE-only) so part of the den-chain latency is
                    # covered; the v transposes must come after fin3[1]
                    # because they share the "sc" psum ring with bc
                    fin3[0]()
                    rope_evac(ps_k2, kT[:, (scb + 1) * CH:(scb + 2) * CH],
                              (scb + 1) * CH)
                    fin3[1]()
                    vproj_finish(ps_v2, scb + 1)
                    o_sbs = [o0, o1, o2, o3]

                    for st in range(CH // 128):
                        for hcp in range(4):
                            opart_pair(o_sbs, st, hcp, rs_ins[scb])
                    if scb < NCH - 2:
                        # defer: trigger only after the NEXT chunk's ht
                        # prefetch has landed (no DMA contention)
                        pending_rs[0] = scb
                    else:
                        emit_rs(scb)
                else:
                    # ---- last chunk: s-split into SUBW sub-chunks so the
                    # final ReduceScatters pipeline against compute ----
                    def emit_rs3(i, row_off):
                        nc.gpsimd.collective_compute(
                            "ReduceScatter",
                            mybir.AluOpType.add,
                            replica_groups=[list(range(NCORES))],
                            ins=[rs3_ins[i].opt()],
                            outs=[rs3_outs[i].opt()],
                        )
                        nc.gpsimd.dma_start(
                            out_d[scb][row_off:
                                       row_off + SUBW[i] // NCORES, :],
                            rs3_outs[i][:])

                    cb0 = scb * CH
                    W0 = SUBW[0]
                    per0 = max(1, KT // ((cb0 + W0) // 128))
                    ps_q2 = psP.tile([128, CH], F32, tag="proj")
                    h0, f0 = attention(
                        0, cb0, W0, 0, qcs[0],
                        proj_thunks([(wq_ts[2], ps_q2)], ht_t, per0))
                    qcs[2] = qcp.tile([128, CH], BF16, tag="qc", name="qc2")
                    rope_evac(ps_q2, qcs[2][:], scb * CH)
                    ps_q3 = psP.tile([128, CH], F32, tag="proj")
                    h1, f1 = attention(
                        1, cb0, W0, 0, qcs[1],
                        weave(f0,
                              proj_thunks([(wq_ts[3], ps_q3)], ht_t, per0)))
                    qcs[3] = qcp.tile([128, CH], BF16, tag="qc", name="qc3")
                    rope_evac(ps_q3, qcs[3][:], scb * CH)
                    h2, f2 = attention(2, cb0, W0, 0, qcs[2],
                                       weave(f1, []))
                    h3, f3 = attention(3, cb0, W0, 0, qcs[3],
                                       weave(f2, []))
                    prev_o, prev_f3 = [h0, h1, h2, h3], f3

                    qo = W0
                    row_off = 0
                    for i in range(1, len(SUBW)):
                        cb = scb * CH + qo
                        W = SUBW[i]
                        pw = SUBW[i - 1]
                        pairs = [
                            (lambda st=st, hcp=hcp, po=prev_o, ii=i - 1:
                             opart_pair(po, st, hcp, rs3_ins[ii]))
                            for st in range(pw // 128) for hcp in range(4)]
                        h0, f0 = attention(
                            0, cb, W, qo, qcs[0],
                            [prev_f3[0], prev_f3[1]] + pairs[0:6])
                        h1, f1 = attention(
                            1, cb, W, qo, qcs[1], weave(f0, pairs[6:]))
                        emit_rs3(i - 1, row_off)
                        row_off += pw // NCORES
                        h2, f2 = attention(2, cb, W, qo, qcs[2],
                                           weave(f1, []))
                        h3, f3 = attention(3, cb, W, qo, qcs[3],
                                           weave(f2, []))
                        prev_o, prev_f3 = [h0, h1, h2, h3], f3
                        qo += W

                    prev_f3[0]()
                    prev_f3[1]()
                    li = len(SUBW) - 1
                    for st in range(SUBW[li] // 128):
                        for hcp in range(4):
                            opart_pair(prev_o, st, hcp, rs3_ins[li])
                    emit_rs3(li, row_off)

                ht_t = ht_next

    nc.compile()
    return nc


def _deinterleave(w):
    # per 128-col head block: [even cols, odd cols]
    hid, cols = w.shape
    nh = cols // DH
    w = w.reshape(hid, nh, DH)
    w = np.concatenate([w[:, :, 0::2], w[:, :, 1::2]], axis=2)
    return w.reshape(hid, cols)


def _prep_inputs(hidden_states, cos, sin, position_ids, attention_mask,
                 wq, wk, wv, wo):
    h = np.asarray(hidden_states, dtype=np.float32)[0]          # [S, HID]
    ht = np.ascontiguousarray(h.T)                              # [HID, S]
    ht4 = np.ascontiguousarray(
        ht.reshape(HID, NCH, CH).transpose(1, 0, 2)).astype(NPBF16)

    pos = np.asarray(position_ids)[0].astype(np.int64)
    ct = np.asarray(cos, dtype=np.float32)[pos].T               # [64, S]
    st = np.asarray(sin, dtype=np.float32)[pos].T
    cos2 = np.ascontiguousarray(np.concatenate([ct, ct], axis=0)).astype(NPBF16)
    sins = np.ascontiguousarray(np.concatenate([st, -st], axis=0)).astype(NPBF16)

    scale = 1.0 / np.sqrt(np.float32(DH))
    wq_p = (_deinterleave(np.asarray(wq, dtype=np.float32)) * scale)
    wk_p = _deinterleave(np.asarray(wk, dtype=np.float32))
    wv_p = np.asarray(wv, dtype=np.float32)
    wo_p = np.asarray(wo, dtype=np.float32).reshape(NH, DH, HID)

    # 0/1 bf16 masks for diagonal t-tiles: mask_i[p, c] = (p + 128*i <= c)
    p = np.arange(128)[:, None]
    c = np.arange(CH)[None, :]
    mask = np.stack([(p + 128 * i <= c) for i in range(4)]).astype(NPBF16)

    in_maps = []
    for core in range(NCORES):
        wq_c = wq_p[:, core * G * DH:(core + 1) * G * DH]       # [HID, 512]
        wq_c = np.ascontiguousarray(
            wq_c.reshape(KT, 128, G, DH).transpose(2, 1, 0, 3).reshape(
                G, 128, KT * DH)).astype(NPBF16)
        wk_c = np.ascontiguousarray(
            wk_p[:, core * DH:(core + 1) * DH].reshape(KT, 128, DH)
            .transpose(1, 0, 2).reshape(128, KT * DH)).astype(NPBF16)
        wv_c = np.ascontiguousarray(
            wv_p[:, core * DH:(core + 1) * DH].reshape(KT, 128, DH)
            .transpose(1, 0, 2).reshape(128, KT * DH)).astype(NPBF16)
        wo_c = np.ascontiguousarray(
            wo_p[core * G:(core + 1) * G]).astype(NPBF16)       # [G,128,HID]
        in_maps.append({
            "ht": ht4, "wq": wq_c, "wk": wk_c, "wv": wv_c, "wo": wo_c,
            "cos2": cos2, "sins": sins, "mask": mask,
        })
    return in_maps


def kernel(hidden_states, cos, sin, position_ids, attention_mask,
           wq, wk, wv, wo, **run_kwargs):
    if "nc" not in _CACHED:
        _CACHED["nc"] = build_kernel()
    nc = _CACHED["nc"]
    in_maps = _prep_inputs(hidden_states, cos, sin, position_ids,
                           attention_mask, wq, wk, wv, wo)
    res = bass_utils.run_bass_kernel_spmd(
        nc, in_maps, core_ids=list(range(NCORES)), **run_kwargs)
    # res[c]["out"] is [NCH, 64, HID] bf16: chunks 0-2 give rows
    # [c*64, (c+1)*64); chunk 3 rows 0-47 are the 384-col sub-chunk
    # (rank stride 48) and rows 48-63 the 128-col one (stride 16)
    full = np.empty((S, HID), dtype=np.float32)
    for c in range(NCORES):
        p = np.asarray(res.results[c]["out"], dtype=np.float32)
        for jj in range(NCH - 1):
            full[jj * CH + c * SC2:jj * CH + (c + 1) * SC2] = p[jj]
        base = (NCH - 1) * CH
        sub_off = 0
        row_off = 0
        for w in (256, 128, 128):
            rw = w // NCORES
            full[base + sub_off + rw * c:base + sub_off + rw * (c + 1)] = \
                p[NCH - 1][row_off:row_off + rw]
            sub_off += w
            row_off += rw
    out = full.reshape(1, S, HID)
    if run_kwargs:
        _CACHED["last_result"] = res
    return out
